# revision 40
# baseline (speedup 1.0000x reference)
"""MinGPT forward pass on 8 Trainium2 NeuronCores (Bass/Tile).

Sharding: core pair (2b, 2b+1) owns batch b. Within a pair, tensor
parallelism: core t of the pair owns attention heads t*8..t*8+7 and MLP
hidden units t*2048..(t+1)*2048, plus sequence rows t*512..(t+1)*512 of
the residual stream h (feature-major [E, rows] layout on chip).

Per layer:  LN1 -> AllGather(x pair-wide, split in row-halves) -> QKV
         -> causal attention (own heads) -> masked ReduceScatter of o
            (split 3-way on head groups) -> h += o -> LN2 -> fc1/gelu
         -> fc2 (row-halves) -> h += mlp.
Final LN + vocab head emit row-major logits for the core's own rows.

Key structure (v2 — tuned from the 5.26ms baseline trace):
- softmax denominator comes free out of the AV matmul via a ones-column
  appended to V; 1/den is computed as Exp(-Ln(den)) on the Act engine
  (Ln+Exp live in one activation table with the scores' Exp, so no
  table thrash), replacing the 3.4us-per-call DVE iterative reciprocal
  that used to gate PSUM bank recycling.
- attention is causally trimmed: per 128-kv tile only the valid query
  range is computed (scores, exp, AV all shrink ~25%), and the two
  heads of a pair run their score matmuls CONCURRENTLY in the PE array
  via row-tiling (K=64 each, partitions 0-63 / 64-127).
- causal masking is a single [128,128] triangular 0/1 multiply per
  diagonal tile (the rest of each tile is fully valid after trimming).
- o-normalize: one unmasked DVE multiply per (head, qgroup); the
  hmsk-masked copies for the pair-exchange ReduceScatter run on the
  otherwise idle GpSimd engine.
- LayerNorm rstd = Exp(-0.5*Ln(var+eps)) folded into the partition
  broadcast (PE matmul with a -0.5 row), removing the DVE reciprocal
  and the sqrt-table load; only 2 Act table loads/layer remain (gelu
  and back).
- fc2 residual+bias is one fused scalar_tensor_tensor on DVE.
- weight streams are 4-deep; wv is double-buffered and prefetched a
  layer ahead; the gathered-x unload is a single 4D DMA per half, with
  half 0 unloaded during the previous layer's fc2.

All per-core specialization is carried by the *input data* (host-sliced
weights and h0 rows); the Bass program itself is identical on all cores.
"""

import sys

sys.path.insert(0, "/opt/trn_rl_repo")

import numpy as np
import ml_dtypes

import concourse.bass as bass
import concourse.bacc as bacc
import concourse.mybir as mybir
from concourse import tile
from concourse.bass_utils import run_bass_kernel_spmd

F32 = mybir.dt.float32
AF = mybir.ActivationFunctionType
OP = mybir.AluOpType

B, S, E, H, D, L, V = 4, 1024, 1024, 16, 64, 12, 1024
NCORES = 8
ROWS = 512          # residual-stream rows owned per core
HL = 8              # heads per core
EPS = 1e-5

# matmul precision mode: "bf16" (fastest) or "f32r" (fp32 storage,
# reduced-precision PE mode, ~1.5x slower than bf16, much more accurate)
MM_MODE = "bf16"

LAST_EXEC_NS = None
LAST_RESULTS = None


def _mmw_dt():
    return mybir.dt.bfloat16 if MM_MODE == "bf16" else mybir.dt.float32r


def _act_store_dt():
    # storage dtype for activation tiles that feed matmuls
    return mybir.dt.bfloat16 if MM_MODE == "bf16" else F32


def _mm(ap):
    # view an activation AP with the dtype the PE should use
    if MM_MODE == "f32r":
        return ap.bitcast(mybir.dt.float32r)
    return ap


def build_nc(n_layers=L):
    MMW = _mmw_dt()
    ADT = _act_store_dt()

    nc = bacc.Bacc(num_devices=NCORES)



    # ---- DRAM parameters (host pre-tiled, see kernel()) ----
    h0_d = nc.dram_tensor("h0", [128, 8 * ROWS], F32, kind="ExternalInput")
    wq_d = nc.dram_tensor("wq", [L, 128, 4096], MMW, kind="ExternalInput")
    wk_d = nc.dram_tensor("wk", [L, 128, 4096], MMW, kind="ExternalInput")
    wv_d = nc.dram_tensor("wv", [L, 128, 4096], MMW, kind="ExternalInput")
    bq_d = nc.dram_tensor("bq", [128, L * 4], F32, kind="ExternalInput")
    bk_d = nc.dram_tensor("bk", [128, L * 4], F32, kind="ExternalInput")
    bv_d = nc.dram_tensor("bv", [1, L * 512], F32, kind="ExternalInput")
    f1w_d = nc.dram_tensor("fc1w", [L, 128, 32768], MMW, kind="ExternalInput")
    f1b_d = nc.dram_tensor("fc1b", [128, L * 32], F32, kind="ExternalInput")
    f2w_d = nc.dram_tensor("fc2w", [L, 128, 32768], MMW, kind="ExternalInput")
    f2b_d = nc.dram_tensor("fc2b", [128, L * 8], F32, kind="ExternalInput")
    lnw_d = nc.dram_tensor("lnw", [128, L * 8], F32, kind="ExternalInput")
    lnb_d = nc.dram_tensor("lnb", [128, L * 8], F32, kind="ExternalInput")
    lnfw_d = nc.dram_tensor("lnfw", [128, 8], F32, kind="ExternalInput")
    lnfb_d = nc.dram_tensor("lnfb", [128, 8], F32, kind="ExternalInput")
    hw_d = nc.dram_tensor("headw", [128, 8192], MMW, kind="ExternalInput")
    msk_d = nc.dram_tensor("masks", [128, 128], ADT, kind="ExternalInput")
    hm_d = nc.dram_tensor("hmsk", [128, 2], F32, kind="ExternalInput")
    out_d = nc.dram_tensor("logits", [ROWS, V], F32, kind="ExternalOutput")
    dbga_d = nc.dram_tensor("dbg_a", [128, 8192], ADT, kind="ExternalOutput")
    dbgf_d = nc.dram_tensor("dbg_f", [128, 4096], F32, kind="ExternalOutput")
    KDBG = __import__("os").environ.get("KDBG", "")

    RG = [[0, 1], [2, 3], [4, 5], [6, 7]]

    with tile.TileContext(nc) as tc:
        with (
            tc.tile_pool(name="const", bufs=1) as cpool,
            tc.tile_pool(name="hres", bufs=1) as hpool,
            tc.tile_pool(name="act", bufs=1) as apool,
            tc.tile_pool(name="wgt4", bufs=4) as w3pool,
            tc.tile_pool(name="wbig", bufs=2) as wbpool,
            tc.tile_pool(name="small", bufs=2) as spool,
            tc.tile_pool(name="tmp", bufs=2) as tpool,
            tc.tile_pool(name="tmp1", bufs=1) as t1pool,
            tc.tile_pool(name="exp", bufs=4) as epool,
            # PSUM budget is 8 banks:
            #   "mm" 4 + "av" 2 + "den" 1 + "stB" 1 = 8  (broadcasts
            #   borrow the stats banks, which are idle at those moments)
            tc.tile_pool(name="psmm", bufs=4, space="PSUM") as psmm,
            tc.tile_pool(name="psav", bufs=2, space="PSUM") as psav,
            tc.tile_pool(name="psden", bufs=1, space="PSUM") as psden,
            tc.tile_pool(name="dram", bufs=2, space="DRAM") as dpool,
        ):
            # ---------- constants ----------
            ones_f32 = cpool.tile([128, 1], F32)   # lhsT for LN stats
            nc.vector.memset(ones_f32[:], 1.0)
            ones_row = cpool.tile([1, 128], F32)   # lhsT for partition-bcast
            nc.vector.memset(ones_row[:], 1.0)
            # bcast of -x (softmax den); rows live at partitions 0/32/64 so
            # the lhsT can match the den slot's base partition.
            neg_row = cpool.tile([65, 128], F32)
            nc.vector.memset(neg_row[:], -1.0)
            negh_row = cpool.tile([1, 128], F32)   # bcast of -x/2 (LN rstd)
            nc.vector.memset(negh_row[:], -0.5)

            tri = cpool.tile([128, 128], ADT)      # 0/1 causal triangle
            nc.sync.dma_start(tri[:], msk_d[:])
            bq_sb = cpool.tile([128, L * 4], F32)
            nc.sync.dma_start(bq_sb[:], bq_d[:])
            bk_sb = cpool.tile([128, L * 4], F32)
            nc.sync.dma_start(bk_sb[:], bk_d[:])
            f1b_sb = cpool.tile([128, L * 32], F32)
            nc.sync.dma_start(f1b_sb[:], f1b_d[:])
            f2b_sb = cpool.tile([128, L * 8], F32)
            nc.sync.dma_start(f2b_sb[:], f2b_d[:])
            lnw_sb = cpool.tile([128, L * 8], F32)
            nc.sync.dma_start(lnw_sb[:], lnw_d[:])
            lnb_sb = cpool.tile([128, L * 8], F32)
            nc.sync.dma_start(lnb_sb[:], lnb_d[:])
            lnfw_sb = cpool.tile([128, 8], F32)
            nc.sync.dma_start(lnfw_sb[:], lnfw_d[:])
            lnfb_sb = cpool.tile([128, 8], F32)
            nc.sync.dma_start(lnfb_sb[:], lnfb_d[:])
            hm_sb = cpool.tile([128, 2], F32)
            nc.sync.dma_start(hm_sb[:], hm_d[:])

            # ---------- residual stream (persistent) ----------
            h_all = hpool.tile([128, 8 * ROWS], F32)   # col et*512+r
            nc.sync.dma_start(h_all[:], h0_d[:])
            if KDBG == "h0":
                nc.sync.dma_start(dbgf_d[:], h_all[:])

            def h_t(et):
                return h_all[:, et * ROWS:(et + 1) * ROWS]

            # ---------- augmented V (persistent; ones col per 65-block) ----
            # col = kt*520 + h*65 + j;  j==64 is the ones column that makes
            # the AV matmul also produce the softmax denominator in row 64.
            va = cpool.tile([128, 8 * 520], ADT)
            va_ones = va[:].rearrange("p (k h c) -> p k h c", k=8, h=8)[:, :, :, 64:65]
            nc.vector.memset(va_ones, 1.0)

            # wv is double-buffered: layer l+1's weights stream in while
            # layer l computes.  The vocab head reuses the same buffers.
            wv_cur = wbpool.tile([128, 4096], MMW, tag="wv", name="wv0")
            nc.sync.dma_start(wv_cur[:], wv_d[0][:])

            RH = 256  # row-half within my 512 rows

            # ---------- layernorm (feature-major), split into stats+tail --
            def ln_stats(lo, W, et_order=range(8)):
                """PE-chain sum(h) and sum(h^2) over E for rows [lo, lo+W)."""
                st1 = psden.tile([1, W], F32, tag="den", name="st1")
                st2 = psden.tile([1, W], F32, tag="stB", name="st2")
                for i, et in enumerate(et_order):
                    hc = h_all[:, et * ROWS + lo: et * ROWS + lo + W]
                    sq = tpool.tile([128, W], F32, tag="ln1", name="sq")
                    nc.gpsimd.tensor_mul(sq[:], hc, hc)
                    nc.tensor.matmul(st1[:], ones_f32[:], hc,
                                     start=(i == 0), stop=(i == 7))
                    nc.tensor.matmul(st2[:], ones_f32[:], sq[:],
                                     start=(i == 0), stop=(i == 7))
                return st1, st2

            def ln_tail(st1, st2, w_ap, b_ap, xn_all, lo, W):
                mean = spool.tile([1, W], F32, tag="mean", bufs=1)
                nc.vector.tensor_scalar_mul(mean[:], st1[:], 1.0 / E)
                msq = spool.tile([1, W], F32, tag="msq", bufs=1)
                # msq = st2/E + EPS  (fold the LN epsilon in here)
                nc.vector.tensor_scalar(msq[:], st2[:], 1.0 / E, EPS,
                                        OP.mult, OP.add)
                var = spool.tile([1, W], F32, tag="denB", bufs=2, name="var")
                nc.vector.tensor_mul(var[:], mean[:], mean[:])
                nc.vector.tensor_sub(var[:], msq[:], var[:])
                lnv = spool.tile([1, W], F32, tag="msq", bufs=1, name="lnv")
                nc.scalar.activation(lnv[:], var[:], AF.Ln)
                # broadcast C=mean and A=rstd=exp(-0.5*ln var) along
                # partitions; (h - mean) starts before rstd is computed.
                A_sb = t1pool.tile([128, 2 * W], ADT, tag="lnA")
                bc2 = psden.tile([128, W], F32, tag="stB", name="bc2")
                nc.tensor.matmul(bc2[:], ones_row[:], mean[:])
                nc.scalar.copy(A_sb[:, W:2 * W], bc2[:])
                bc = psden.tile([128, W], F32, tag="den", name="bc")
                nc.tensor.matmul(bc[:], negh_row[:], lnv[:])
                nc.scalar.activation(A_sb[:, 0:W], bc[:], AF.Exp)
                for et in range(8):
                    t = tpool.tile([128, W], F32, tag="ln1")
                    nc.vector.tensor_sub(t[:], h_all[:, et * ROWS + lo:
                                                     et * ROWS + lo + W],
                                         A_sb[:, W:2 * W])
                    nc.gpsimd.tensor_mul(t[:], t[:], A_sb[:, 0:W])
                    nc.vector.tensor_scalar(
                        xn_all[:, et * ROWS + lo: et * ROWS + lo + W], t[:],
                        w_ap(et), b_ap(et), OP.mult, OP.add)

            def layer_norm(w_ap, b_ap, xn_all, rh, rn=1, et_order=range(8)):
                lo, W = rh * RH, rn * RH
                st1, st2 = ln_stats(lo, W, et_order)
                ln_tail(st1, st2, w_ap, b_ap, xn_all, lo, W)

            def ln1_half(l, xn, rh, stats=None):
                """LN1 row-half rh of layer l, ship it, launch its AllGather."""
                if stats is None:
                    stats = ln_stats(rh * RH, RH)
                ln_tail(stats[0], stats[1],
                        lambda et: lnw_sb[:, l * 8 + et:l * 8 + et + 1],
                        lambda et: lnb_sb[:, l * 8 + et:l * 8 + et + 1],
                        xn, rh * RH, RH)
                cc1i = dpool.tile([1024, RH], ADT, tag=f"cc1i{rh}")
                cc1o = dpool.tile([2048, RH], ADT, tag=f"cc1o{rh}")
                # one 3D DMA ships all 8 feature blocks
                nc.sync.dma_start(
                    cc1i[:].rearrange("(e p) r -> p e r", e=8),
                    xn[:].rearrange("p (e r) -> p e r", e=8)[
                        :, :, rh * RH: rh * RH + RH])
                nc.gpsimd.collective_compute(
                    "AllGather", OP.bypass, replica_groups=RG,
                    ins=[cc1i[:].opt()], outs=[cc1o[:].opt()])
                return cc1o

            def unload_xf_half(xf, cc1o, rh):
                """cc1o[rk*1024+et*128+p, r] -> xf[:, et*1024+rk*512+rh*RH+r]
                as one 3D DMA per rank block."""
                for rk in range(2):
                    dst = xf[:].rearrange("p (e k c) -> p e k c", e=8, k=2)[
                        :, :, rk, rh * RH: rh * RH + RH]
                    src = cc1o[rk * 1024:(rk + 1) * 1024, :].rearrange(
                        "(e p) r -> p e r", e=8)
                    nc.sync.dma_start(dst, src)

            # ---- prologue: LN1 + AllGather halves for layer 0 ----
            xn_next = apool.tile([128, 8 * ROWS], ADT, tag="xn")
            cc1o_pair = [ln1_half(0, xn_next, 0), ln1_half(0, xn_next, 1)]
            if KDBG == "xn":
                nc.sync.dma_start(dbga_d[:, 0:4096], xn_next[:])

            xf = apool.tile([128, 8192], ADT, tag="xf")
            unload_xf_half(xf, cc1o_pair[0], 0)

            # ================= layers =================
            for l in range(n_layers):
                # xf half 0 for this layer was unloaded during the previous
                # layer's fc2 (or in the prologue); half 1 is unloaded after
                # the row-half-0 QKV work below.
                if l == 0 and KDBG == "xf":
                    nc.sync.dma_start(dbga_d[:], xf[:])

                # ---- QKV (own 8 heads, all 1024 rows), by row-quarter,
                #      with attention interleaved into the rh1 sweep so the
                #      PE and Act stay busy together ----
                # q_all/k_all: feature-major [512f, 1024r]; col hp*1024 + r
                q_all = apool.tile([128, 4096], ADT, tag="q")
                k_all = apool.tile([128, 4096], ADT, tag="k")
                bv_row = spool.tile([1, 512], F32, tag="bvrow", bufs=1)
                nc.sync.dma_start(bv_row[:], bv_d[0:1, l * 512:(l + 1) * 512])
                vb_ps = psden.tile([128, 512], F32, tag="den", name="vb_ps")
                nc.tensor.matmul(vb_ps[:], ones_row[:], bv_row[:])
                vb = t1pool.tile([128, 512], ADT, tag="vb")
                nc.scalar.copy(vb[:], vb_ps[:])
                vb_v = vb[:].rearrange("p (h c) -> p h c", h=8)

                def qk_quarter(w_d, b_sb, dst, mq, rh):
                    wt = w3pool.tile([128, 1024], MMW, tag="wqk", name="wt",
                                     bufs=3)
                    nc.sync.dma_start(
                        wt[:], w_d[l][:, mq * 1024:(mq + 1) * 1024])
                    for rk in range(2):
                        ps = psmm.tile([128, RH], F32, tag="mm", name="ps")
                        base = rk * 512 + rh * RH
                        for et in range(8):
                            nc.tensor.matmul(
                                ps[:], wt[:, et * 128:(et + 1) * 128],
                                _mm(xf[:, et * 1024 + base:
                                       et * 1024 + base + RH]),
                                start=(et == 0), stop=(et == 7))
                        nc.vector.tensor_scalar_add(
                            dst[:, mq * 1024 + base: mq * 1024 + base + RH],
                            ps[:], b_sb[:, l * 4 + mq:l * 4 + mq + 1])

                def v_block(rt):
                    ps = psmm.tile([128, 512], F32, tag="mm", name="ps")
                    for et in range(8):
                        nc.tensor.matmul(
                            ps[:], _mm(xf[:, et * 1024 + rt * 128:
                                          et * 1024 + (rt + 1) * 128]),
                            wv_cur[:, et * 512:(et + 1) * 512],
                            start=(et == 0), stop=(et == 7))
                    va_view = va[:, rt * 520:(rt + 1) * 520].rearrange(
                        "p (h c) -> p h c", h=8)[:, :, 0:64]
                    ps_view = ps[:].rearrange("p (h c) -> p h c", h=8)
                    nc.vector.tensor_add(va_view, ps_view, vb_v)

                # ---- causal attention machinery, own 8 heads ----
                # o_q[tb][qg]: [128, 2048] = o^T[f=hp*128+p, r=c], normalized
                # and pre-masked by hmsk[:, tb] (nonzero only for tb == my t)
                o_q = [[apool.tile([128, 2048], ADT, tag=f"o{tb}{qg}",
                                   name=f"o{tb}{qg}") for qg in range(2)]
                       for tb in range(2)]
                # the o-exchange ReduceScatter is split on head groups
                # (hp 0-1 / hp 2 / hp 3): earlier groups fly while later
                # heads still compute; only the last small RS is exposed.
                cc2i = [dpool.tile([1024 if g == 0 else 512, ROWS], ADT,
                                   tag=f"cc2i{g}", name=f"cc2i{g}")
                        for g in range(3)]
                cc2o = [dpool.tile([512 if g == 0 else 256, ROWS], ADT,
                                   tag=f"cc2o{g}", name=f"cc2o{g}")
                        for g in range(3)]
                RS_ETS = ((0, 1, 4, 5), (2, 6), (3, 7))

                o_ps_cur = {}
                pend = []
                fin_pend = []
                state = {"fin": 0}

                def kt_geom(qg, kt):
                    """(qoff, N, exoff, diag) for kv tile kt in query group
                    qg.  exoff is the (packed) column offset of the tile in
                    its exm buffer."""
                    if qg == 1 and kt < 4:
                        return 0, 512, kt * 512, False
                    dj = kt - (4 if qg == 1 else 0)
                    # diag chunks pack tiles tightly: offsets 0,512,896,1152
                    exoff = (0, 512, 896, 1152)[dj]
                    return dj * 128, 512 - dj * 128, exoff, True

                def emit_scores_pair(hp, qg, kt0, exA, exB):
                    """Scores+exp for BOTH heads of pair hp, 4 kv tiles.
                    The two heads' K=64 matmuls run concurrently in the PE
                    array (row groups 0-63 / 64-127)."""
                    kb = hp * 1024
                    qb = hp * 1024 + qg * 512
                    for j in range(4):
                        kt = kt0 + j
                        qoff, N, xo, diag = kt_geom(qg, kt)
                        scA = psmm.tile([128, 512], F32, tag="mm", name="scA")
                        scB = psmm.tile([128, 512], F32, tag="mm", name="scB")
                        nc.tensor.matmul(
                            scA[:, 0:N],
                            _mm(k_all[0:64, kb + kt * 128: kb + (kt + 1) * 128]),
                            _mm(q_all[0:64, qb + qoff: qb + 512]))
                        nc.tensor.matmul(
                            scB[:, 0:N],
                            _mm(k_all[64:128, kb + kt * 128: kb + (kt + 1) * 128]),
                            _mm(q_all[64:128, qb + qoff: qb + 512]))
                        for sc, exm in ((scA, exA), (scB, exB)):
                            nc.scalar.activation(
                                exm[:, xo: xo + N],
                                sc[:, 0:N], AF.Exp, scale=0.125)
                        if diag:
                            for exm in (exA, exB):
                                nc.vector.tensor_mul(
                                    exm[:, xo: xo + 128],
                                    exm[:, xo: xo + 128],
                                    tri[:])

                def flush_fin():
                    """Deferred finalize for a whole head-pair (4 chains):
                    ln(den) x4, broadcast -ln(den), exp, masked normalize,
                    ship.  The Ln and Exp calls cluster so the exp<->ln
                    table reload is paid twice per head-pair instead of
                    twice per chain."""
                    for (hp, qg, ho, h, o_u, lnden_ap) in fin_pend:
                        nc.scalar.activation(lnden_ap, lnden_ap, AF.Ln)
                    for (hp, qg, ho, h, o_u, lnden_ap) in fin_pend:
                        bp = lnden_ap.base_partition()
                        r_ps = psden.tile([64, 512], F32,
                                          tag="den" if (h + qg) % 2 == 0
                                          else "stB", name="r_ps")
                        nc.tensor.matmul(r_ps[:],
                                         neg_row[bp:bp + 1, 0:64], lnden_ap)
                        r_sb = tpool.tile([64, 512], ADT, tag="rsb")
                        nc.scalar.activation(r_sb[:], r_ps[:], AF.Exp)
                        for tb in range(2):
                            # o_q = (o_u * hmsk) * r  fused on DVE.  All
                            # SBUF inputs must share a base partition, so
                            # always take the partition-0 hmsk slice (its
                            # values repeat on every partition).
                            nc.vector.scalar_tensor_tensor(
                                o_q[tb][qg][ho:ho + 64, hp * 512:(hp + 1) * 512],
                                o_u[:], hm_sb[0:64, tb:tb + 1], r_sb[:],
                                OP.mult, OP.mult)
                            if ho == 64:  # both heads of hp done: ship
                                if hp < 2:
                                    row = qg * 512 + tb * 256 + hp * 128
                                    g = 0
                                else:
                                    row = qg * 256 + tb * 128
                                    g = hp - 1
                                nc.sync.dma_start(
                                    cc2i[g][row:row + 128, :],
                                    o_q[tb][qg][:, hp * 512:(hp + 1) * 512])
                        state["fin"] += 1
                        if state["fin"] == 8:    # heads 0-3 complete
                            rs_group(0)
                            h_accum_group(0)
                        elif state["fin"] == 12:  # heads 4-5 complete
                            rs_group(1)
                            h_accum_group(1)
                    fin_pend.clear()

                def emit_av_head(e):
                    hp, qg, kt0, ho, exm, first, last = e
                    h = 2 * hp + (1 if ho else 0)
                    key = (h, qg)
                    if first:
                        o_ps_cur[key] = psav.tile([65, 512], F32, tag="av",
                                                  name="o_ps")
                    o_ps = o_ps_cur[key]
                    for j in range(4):
                        kt = kt0 + j
                        qoff, N, xo, diag = kt_geom(qg, kt)
                        nc.tensor.matmul(
                            o_ps[:, qoff:512],
                            _mm(va[:, kt * 520 + h * 65: kt * 520 + h * 65 + 65]),
                            _mm(exm[:, xo: xo + N]),
                            start=(first and j == 0), stop=(last and j == 3))
                    if not last:
                        return
                    # stage 1 of finalize: copy den and o out of PSUM (plain
                    # copies are table-neutral w.r.t. exp), freeing the AV
                    # bank immediately.  The rest is clustered in flush_fin.
                    # den slots sit at partitions 0/32/64 (+ a second tile)
                    # so they are legal matmul-rhs base partitions.
                    cslot = len(fin_pend)
                    if cslot == 0:
                        state["dA"] = spool.tile([65, 512], F32, tag="denA",
                                                 name="denA")
                        state["dB"] = spool.tile([1, 512], F32, tag="denB",
                                                 name="denB")
                    if cslot < 3:
                        lnden = state["dA"][32 * cslot:32 * cslot + 1, :]
                    else:
                        lnden = state["dB"][0:1, :]
                    nc.scalar.copy(lnden, o_ps[64:65, :])
                    o_u = tpool.tile([64, 512], ADT, tag="ou", bufs=4)
                    nc.vector.tensor_copy(o_u[:], o_ps[0:64, :])
                    fin_pend.append((hp, qg, ho, h, o_u, lnden))
                    if len(fin_pend) == 4:
                        flush_fin()

                def rs_group(g):
                    nc.gpsimd.collective_compute(
                        "ReduceScatter", OP.add, replica_groups=RG,
                        ins=[cc2i[g][:].opt()], outs=[cc2o[g][:].opt()])

                def h_accum_group(g):
                    # contiguous et runs -> one DMA + one Pool add per run
                    ets = RS_ETS[g]
                    runs = [(0, 2), (2, 2)] if g == 0 else [(0, 1), (1, 1)]
                    for i0, n in runs:
                        ot = tpool.tile([128, n * ROWS], ADT, tag="ot",
                                        name="ot")
                        nc.sync.dma_start(
                            ot[:].rearrange("p (e r) -> p e r", e=n),
                            cc2o[g][i0 * 128:(i0 + n) * 128, :].rearrange(
                                "(e p) r -> p e r", e=n))
                        et = ets[i0]
                        nc.gpsimd.tensor_add(
                            h_all[:, et * ROWS:(et + n) * ROWS],
                            h_all[:, et * ROWS:(et + n) * ROWS], ot[:])

                # software pipeline over chunks; AV trails scores/exp so the
                # PE never waits on Act's exp backlog.
                def pump(chunk=None):
                    if chunk is not None:
                        hp, qg, kt0, first, last = chunk
                        diag = (qg == 0) or (kt0 >= 4)
                        etag = "exmd" if diag else "exmf"
                        ecols = 1280 if diag else 2048
                        ebufs = 4 if diag else 2
                        exA = epool.tile([128, ecols], ADT, tag=etag,
                                         name="exA", bufs=ebufs)
                        exB = epool.tile([128, ecols], ADT, tag=etag,
                                         name="exB", bufs=ebufs)
                        emit_scores_pair(hp, qg, kt0, exA, exB)
                        pend.append((hp, qg, kt0, 0, exA, first, last))
                        pend.append((hp, qg, kt0, 64, exB, first, last))
                        while len(pend) > 6:
                            emit_av_head(pend.pop(0))
                    elif pend:
                        emit_av_head(pend.pop(0))

                def head_pair_chunks(hp):
                    pump((hp, 0, 0, True, True))
                    pump((hp, 1, 0, True, False))
                    pump((hp, 1, 4, False, True))

                # rh0 sweep: Q/K quarters + V blocks of rows 0-255 (+512-767)
                for (w_d, b_sb, dst) in ((wq_d, bq_sb, q_all),
                                         (wk_d, bk_sb, k_all)):
                    for mq in range(4):
                        qk_quarter(w_d, b_sb, dst, mq, 0)
                for rt in (0, 1, 4, 5):
                    v_block(rt)
                # rows half 1 arrive: one 4D DMA
                unload_xf_half(xf, cc1o_pair[1], 1)
                for rt in (2, 3, 6, 7):
                    v_block(rt)
                # prefetch next layer's V weights into the spare buffer
                if l + 1 < n_layers:
                    wv_nxt = w_nxt = wbpool.tile([128, 4096], MMW, tag="wv",
                                                 name="wv_nxt")
                    nc.sync.dma_start(wv_nxt[:], wv_d[l + 1][:])
                else:
                    wv_nxt = None
                # rh1 sweep, attention rides right behind each head pair
                for hp in range(4):
                    qk_quarter(wq_d, bq_sb, q_all, hp, 1)
                    qk_quarter(wk_d, bk_sb, k_all, hp, 1)
                    head_pair_chunks(hp)
                while pend:
                    pump()
                if fin_pend:
                    flush_fin()
                rs_group(2)
                h_accum_group(2)

                if l == 0 and KDBG == "q":
                    nc.sync.dma_start(dbga_d[:, 0:4096], q_all[:])
                if l == 0 and KDBG == "k":
                    nc.sync.dma_start(dbga_d[:, 0:4096], k_all[:])

                if l == 0 and KDBG == "o":
                    for qg in range(2):
                        nc.sync.dma_start(
                            dbga_d[:, qg * 2048:(qg + 1) * 2048], o_q[1][qg][:])

                # ---- LN2 (same ln params, as in source) ----
                xn2 = apool.tile([128, 8 * ROWS], ADT, tag="xn")
                layer_norm(lambda et: lnw_sb[:, l * 8 + et:l * 8 + et + 1],
                           lambda et: lnb_sb[:, l * 8 + et:l * 8 + et + 1],
                           xn2, 0, rn=2, et_order=(0, 1, 4, 5, 2, 6, 3, 7))

                # ---- MLP fc1 (full hidden, own 512 rows) ----
                h1 = apool.tile([128, 16384], ADT, tag="big")   # col mh*512+r
                for mh in range(32):
                    wt = w3pool.tile([128, 1024], MMW, tag="wf1", bufs=3)
                    nc.sync.dma_start(wt[:],
                                      f1w_d[l][:, mh * 1024:(mh + 1) * 1024])
                    ps = psmm.tile([128, 512], F32, tag="mm")
                    for et in range(8):
                        nc.tensor.matmul(
                            ps[:], wt[:, et * 128:(et + 1) * 128],
                            _mm(xn2[:, et * ROWS:(et + 1) * ROWS]),
                            start=(et == 0), stop=(et == 7))
                    nc.scalar.activation(
                        h1[:, mh * 512:(mh + 1) * 512], ps[:], AF.Gelu,
                        bias=f1b_sb[:, l * 32 + mh:l * 32 + mh + 1])

                # ---- fc2 + residual by row-half; each finished half runs
                #      LN1 of the next layer and launches its AllGather so
                #      the collective rides under remaining fc2 compute ----
                if l + 1 < n_layers:
                    xn_next = apool.tile([128, 8 * ROWS], ADT, tag="xn")
                for rh in range(2):
                    do_ln = l + 1 < n_layers
                    if do_ln:
                        st1 = psden.tile([1, RH], F32, tag="den", name="st1")
                        st2 = psden.tile([1, RH], F32, tag="stB", name="st2")
                    for mo in range(8):
                        ps = psmm.tile([128, RH], F32, tag="mm")
                        for half in range(2):
                            wt = w3pool.tile([128, 2048], MMW, tag="wf2",
                                             bufs=3)
                            nc.sync.dma_start(
                                wt[:], f2w_d[l][:, mo * 4096 + half * 2048:
                                                mo * 4096 + (half + 1) * 2048])
                            for kt in range(16):
                                kg = half * 16 + kt
                                nc.tensor.matmul(
                                    ps[:], wt[:, kt * 128:(kt + 1) * 128],
                                    _mm(h1[:, kg * 512 + rh * RH:
                                           kg * 512 + rh * RH + RH]),
                                    start=(kg == 0), stop=(kg == 31))
                        hs = h_all[:, mo * ROWS + rh * RH:
                                   mo * ROWS + rh * RH + RH]
                        # h += ps + bias, fused on DVE
                        nc.vector.scalar_tensor_tensor(
                            hs, ps[:], f2b_sb[:, l * 8 + mo:l * 8 + mo + 1],
                            hs, OP.add, OP.add)
                        if do_ln:
                            # LN1(l+1) stats ride right behind each h block
                            sq = tpool.tile([128, RH], F32, tag="ln1",
                                            name="sq")
                            nc.gpsimd.tensor_mul(sq[:], hs, hs)
                            nc.tensor.matmul(st1[:], ones_f32[:], hs,
                                             start=(mo == 0), stop=(mo == 7))
                            nc.tensor.matmul(st2[:], ones_f32[:], sq[:],
                                             start=(mo == 0), stop=(mo == 7))
                    if do_ln:
                        cc1o_pair[rh] = ln1_half(l + 1, xn_next, rh,
                                                 stats=(st1, st2))

                if l + 1 < n_layers:
                    # next layer's xf half 0: by now AG0 (launched mid-fc2)
                    # has landed, so this does not block the sync queue.
                    unload_xf_half(xf, cc1o_pair[0], 0)

                if wv_nxt is not None:
                    wv_cur = wv_nxt

                if l == 0 and KDBG == "hattn":
                    nc.sync.dma_start(dbgf_d[:], h_all[:])
                if l == 0 and KDBG == "h1":
                    nc.sync.dma_start(dbga_d[:], h1[:, 0:8192])
                if l == 0 and KDBG == "hlayer":
                    nc.sync.dma_start(dbgf_d[:], h_all[:])

            # ================= final LN + head =================
            xnf = apool.tile([128, 8 * ROWS], ADT, tag="xn")
            layer_norm(lambda et: lnfw_sb[:, et:et + 1],
                       lambda et: lnfb_sb[:, et:et + 1],
                       xnf, 0, rn=2)
            for vn in range(2):
                hw_sb = wbpool.tile([128, 4096], MMW, tag="wv", name="whd")
                for et in range(8):
                    nc.sync.dma_start(
                        hw_sb[:, et * 512:(et + 1) * 512],
                        hw_d[:, et * 1024 + vn * 512: et * 1024 + vn * 512 + 512])
                for rt in range(4):
                    ps = psmm.tile([128, 512], F32, tag="mm")
                    for et in range(8):
                        nc.tensor.matmul(
                            ps[:],
                            _mm(xnf[:, et * ROWS + rt * 128: et * ROWS + rt * 128 + 128]),
                            hw_sb[:, et * 512:(et + 1) * 512],
                            start=(et == 0), stop=(et == 7))
                    lt = tpool.tile([128, 512], F32, tag="ln1")
                    nc.vector.tensor_copy(lt[:], ps[:])
                    nc.sync.dma_start(out_d[rt * 128:(rt + 1) * 128,
                                            vn * 512:(vn + 1) * 512], lt[:])

    nc.finalize()
    return nc


# ---------------------------------------------------------------------------
#  Host side: shard/pre-tile inputs, run, gather
# ---------------------------------------------------------------------------

def _tile_lhsT(w, m_blk):
    """[1024?, Mtot] -> [128, (Mtot/128/?)*...]: (K,M) -> blocks (mi, et).

    w: [K, M] with K=k_tiles*128. Returns [128, m_blocks*k_tiles*128] where
    col = mi*(k_tiles*128) + et*128 + m  maps to w[et*128+p, mi*128+m].
    """
    Kdim, Mdim = w.shape
    kt, mt = Kdim // 128, Mdim // 128
    # [kt,128,mt,128] -> (mi, et) blocks
    w4 = w.reshape(kt, 128, mt, 128)
    out = np.empty((128, mt * kt * 128), dtype=w.dtype)
    for mi in range(mt):
        blk = w4[:, :, mi, :]                  # [kt, 128p, 128m]
        blk = np.transpose(blk, (1, 0, 2)).reshape(128, kt * 128)
        out[:, mi * kt * 128:(mi + 1) * kt * 128] = blk
    return out


def _tile_rhs(w):
    """(K, N) -> [128, kt*N] with col = et*N + n."""
    Kdim, Ndim = w.shape
    kt = Kdim // 128
    return np.ascontiguousarray(
        np.transpose(w.reshape(kt, 128, Ndim), (1, 0, 2)).reshape(128, kt * Ndim))


def _tile_vec(v, blk=128):
    """(L?, F) with F=ft*128 -> [128, L*ft] col = l*ft + et."""
    if v.ndim == 1:
        v = v[None, :]
    Ldim, F = v.shape
    ft = F // blk
    return np.ascontiguousarray(
        np.transpose(v.reshape(Ldim, ft, blk), (2, 0, 1)).reshape(blk, Ldim * ft))


def kernel(tokens, tok_emb, pos_emb, ln_w, ln_b, qkv_w, qkv_b,
           fc1_w, fc1_b, fc2_w, fc2_b, lnf_w, lnf_b, head_w):
    global LAST_EXEC_NS, LAST_RESULTS
    f32 = np.float32
    tokens = np.asarray(tokens)
    tok_emb = np.asarray(tok_emb, f32)
    pos_emb = np.asarray(pos_emb, f32)
    ln_w = np.asarray(ln_w, f32); ln_b = np.asarray(ln_b, f32)
    qkv_w = np.asarray(qkv_w, f32); qkv_b = np.asarray(qkv_b, f32)
    fc1_w = np.asarray(fc1_w, f32); fc1_b = np.asarray(fc1_b, f32)
    fc2_w = np.asarray(fc2_w, f32); fc2_b = np.asarray(fc2_b, f32)
    lnf_w = np.asarray(lnf_w, f32); lnf_b = np.asarray(lnf_b, f32)
    head_w = np.asarray(head_w, f32)

    mm_np = ml_dtypes.bfloat16 if MM_MODE == "bf16" else f32

    # embedding on host (0.1% of model FLOPs)
    emb = tok_emb[tokens.astype(np.int64)] + pos_emb[None, :S, :]   # [B,S,E]

    # 0/1 causal triangle for the [128,128] diagonal sub-tiles of
    # scores^T [kv, q]: valid iff kv_p <= q_c.
    p = np.arange(128)[:, None]
    c = np.arange(128)[None, :]
    masks = (p <= c).astype(f32)

    in_maps = []
    for core in range(NCORES):
        b, t = core // 2, core % 2
        hs = t * 8          # first head
        # per-core slices
        wq = qkv_w[:, :, hs * D:(hs + 8) * D]                 # [L,1024,512]
        wk = qkv_w[:, :, E + hs * D: E + (hs + 8) * D]
        wv = qkv_w[:, :, 2 * E + hs * D: 2 * E + (hs + 8) * D]
        bq = qkv_b[:, hs * D:(hs + 8) * D]
        bk = qkv_b[:, E + hs * D:E + (hs + 8) * D]
        bv = qkv_b[:, 2 * E + hs * D:2 * E + (hs + 8) * D]

        h0 = emb[b, t * ROWS:(t + 1) * ROWS, :].T             # [1024, 512]

        im = {
            "h0": _tile_rhs(np.ascontiguousarray(h0)).astype(f32),
            "wq": np.stack([_tile_lhsT(wq[l], 128) for l in range(L)]).astype(mm_np),
            "wk": np.stack([_tile_lhsT(wk[l], 128) for l in range(L)]).astype(mm_np),
            "wv": np.stack([_tile_rhs(wv[l]) for l in range(L)]).astype(mm_np),
            "bq": _tile_vec(bq).astype(f32),
            "bk": _tile_vec(bk).astype(f32),
            "bv": np.ascontiguousarray(bv.reshape(1, L * 512)).astype(f32),
            "fc1w": np.stack([_tile_lhsT(fc1_w[l], 128) for l in range(L)]).astype(mm_np),
            "fc1b": _tile_vec(fc1_b).astype(f32),
            "fc2w": np.stack([_tile_lhsT(fc2_w[l], 128) for l in range(L)]).astype(mm_np),
            "fc2b": _tile_vec(fc2_b).astype(f32),
            "lnw": _tile_vec(ln_w).astype(f32),
            "lnb": _tile_vec(ln_b).astype(f32),
            "lnfw": _tile_vec(lnf_w).astype(f32),
            "lnfb": _tile_vec(lnf_b).astype(f32),
            "headw": _tile_rhs(head_w).astype(mm_np),
            "masks": masks.astype(mm_np),
            "hmsk": np.ascontiguousarray(
                np.broadcast_to(np.eye(2, dtype=f32)[t][None, :], (128, 2))),
        }
        in_maps.append(im)

    nc = build_nc()
    res = run_bass_kernel_spmd(nc, in_maps, core_ids=list(range(NCORES)),
                               trace=bool(int(__import__("os").environ.get("KTRACE", "0"))))
    LAST_EXEC_NS = res.exec_time_ns
    LAST_RESULTS = res

    out = np.empty((B, S, V), f32)
    for core in range(NCORES):
        b, t = core // 2, core % 2
        out[b, t * ROWS:(t + 1) * ROWS, :] = res.results[core]["logits"]
    return out
